# revision 26
# baseline (speedup 1.0000x reference)
"""GQA attention block (B=2, N=2048, D=2048, H=16, HKV=4, HD=128) on 8 TRN2 cores.

Sharding: core c -> batch b = c // 4, query-row quarter j = c % 4 (512 rows).
Each core:
  - projects K,V for its row slice (K projection dc-major so matmuls start as
    soon as the first weight chunks land), applies RoPE to K, AllGathers K,V
    within its 4-core batch group (single 1MB collective). The bounce-buffer
    writes are issued from the GpSimd SWDGE queue so they neither wait behind
    the 8MB Wq load on the Sync queue nor block the RoPE copies on the Scalar
    queue -- this fires the AllGather ~30us earlier.
  - Q projection is FUSED with the local-attention stage L: each iteration
    runs [Qproj head i+1 | scores head i | AV head i-1] on the PE, so the
    ScalarE exp and RoPE copies hide inside the 4.2us Qproj block and the
    softmax exp is never on the critical path here.
  - stage R (the three remote rank slots, read from the gathered buffer with
    rank-dependent dynamic DMA offsets, streamed one kv-head ahead): scores
    in groups of THREE key blocks per exp activation (amortizes the ~300
    cycle ACT fixed cost) and the AV burst lags TWO windows behind the
    scores, keeping both semaphore round trips off the critical loop. This
    makes stage R tensor-bound instead of exp-bound.
  - softmax denominator via a ones-column appended to V; per-query
    normalization on DVE; o -> o^T via DMA xbar transpose issued per head
    (no PE cycles, no cold-clock window from a sparse transpose phase).
  - output projection over all heads -> its own 512 output rows (no reduce);
    Wo is prefetched during stage R.
All matmuls bf16 with f32 PSUM accumulation; softmax statistics in f32.
"""

import numpy as np
import ml_dtypes

import concourse.bass as bass
import concourse.mybir as mybir
import concourse.tile as tile
from concourse import bacc, masks
from concourse.bass_utils import run_bass_kernel_spmd

B, N, D = 2, 2048, 2048
H, HKV, HD = 16, 4, 128
NQ = N // 4          # query rows per core
DC = D // 128        # contraction chunks for projections
KB = N // 128        # key blocks
NCORES = 8
SCALE = float(HD) ** -0.5

BF16 = mybir.dt.bfloat16
F32 = mybir.dt.float32
MUL = mybir.AluOpType.mult
ADD = mybir.AluOpType.add
EXP = mybir.ActivationFunctionType.Exp

_cache = {}


def _rope(nc, pool, out, in_psum, cos2_sb, sin2_sb):
    """Rotate-half RoPE with head-dim on partitions.

    cos2_sb = [cos; cos], sin2_sb = [sin; -sin] (128 rows, host-prepared), so
    out = t*cos2 + rot(t)*sin2 where rot swaps the partition halves.
    ScalarE does the PSUM reads; the three DVE multiplies/adds then run
    all-SBUF at the 2x f32 rate.
    """
    rot = pool.tile([128, NQ], F32, name="rope_rot")
    nc.scalar.copy(rot[0:64, :], in_psum[64:128, :])
    nc.scalar.copy(rot[64:128, :], in_psum[0:64, :])
    m1 = pool.tile([128, NQ], F32, name="rope_m1")
    m2 = pool.tile([128, NQ], F32, name="rope_m2")
    nc.vector.tensor_tensor(m1[:], in_psum[:], cos2_sb[:], MUL)
    nc.vector.tensor_tensor(m2[:], rot[:], sin2_sb[:], MUL)
    nc.vector.tensor_tensor(out[:], m1[:], m2[:], ADD)


def _build():
    from contextlib import ExitStack

    nc = bacc.Bacc("TRN2", target_bir_lowering=False, debug=False,
                   num_devices=NCORES)

    xT_d = nc.dram_tensor("xT", [D, NQ], BF16, kind="ExternalInput").ap()
    cosT_d = nc.dram_tensor("cosT", [HD, NQ], F32, kind="ExternalInput").ap()
    sinT_d = nc.dram_tensor("sinT", [HD, NQ], F32, kind="ExternalInput").ap()
    wq_d = nc.dram_tensor("wq", [H, 128, DC, 128], BF16, kind="ExternalInput").ap()
    wk_d = nc.dram_tensor("wk", [HKV, 128, DC, 128], BF16, kind="ExternalInput").ap()
    wv_d = nc.dram_tensor("wv", [DC, 128, HKV * HD], BF16, kind="ExternalInput").ap()
    wo_d = nc.dram_tensor("wo", [H, 128, D], BF16, kind="ExternalInput").ap()
    out_d = nc.dram_tensor("out", [NQ, D], F32, kind="ExternalOutput").ap()

    with tile.TileContext(nc) as tc, ExitStack() as top:
        resident = top.enter_context(tc.tile_pool(name="resident", bufs=1))
        dram = top.enter_context(tc.tile_pool(name="dram", bufs=1, space="DRAM"))

        q_sb = resident.tile([128, H, NQ], BF16)
        k_loc = resident.tile([128, HKV, NQ], BF16)       # roped local K, d-major
        vp_loc = resident.tile([128, HKV * 4, HD + 1], BF16)  # (hk, local kb)
        nc.gpsimd.memset(vp_loc[:, :, HD:HD + 1], 1.0)
        # unnormalized local partials (h, qc), col 128 = partial denominator
        o_part = resident.tile([128, H * 4, HD + 1], BF16)

        # remote K/V slab pools reserved at top level: their SBUF must NOT
        # overlap the proj-scope bytes, so the slab DMAs can start the moment
        # the AllGather completes (during the fused Q+L phase)
        ktpool = top.enter_context(tc.tile_pool(name="ktpool", bufs=3))
        # all remote V, loaded in 3 per-slot batched reads (one per rank slot,
        # covering all 4 kv-heads: 4x fewer DMA descriptors to generate)
        vpr_all = top.enter_context(tc.tile_pool(name="vpra", bufs=1)).tile(
            [128, 3, 4, HKV, HD + 1], BF16)
        nc.gpsimd.memset(vpr_all[:, :, :, :, HD:HD + 1], 1.0)

        # kv bounce: rows 0..511 = roped K (4 heads x 128 d), cols = local n;
        # rows 512..1023 = V (local n rows), cols = 4 heads x 128 channels
        kv_bounce = dram.tile([2 * NQ, NQ], BF16)
        ag_out = dram.tile([2 * NQ * 4, NQ], BF16)

        # -- projection scope: tensors freed after the fused Q+L phase --------
        proj_scope = ExitStack()
        proj = proj_scope.enter_context(tc.tile_pool(name="proj", bufs=1))
        tmp_pool = proj_scope.enter_context(tc.tile_pool(name="ropetmp", bufs=3))
        cos_sb = proj.tile([HD, NQ], F32)
        sin_sb = proj.tile([HD, NQ], F32)
        xts = proj.tile([128, DC, NQ], BF16)
        wq_sb = proj.tile([128, H, DC * 128], BF16)
        xT_r = xT_d.rearrange("(dc p) n -> p dc n", p=128)

        # pt/npool shared by stage L and stage R; entered after the proj pools
        # but they only hold small streaming tiles
        ptpool = proj_scope.enter_context(tc.tile_pool(name="ptpool", bufs=2))
        npool = proj_scope.enter_context(tc.tile_pool(name="npool", bufs=6))

        # ---------------- KV projection + RoPE(K) + bounce-out ----------------
        with ExitStack() as ph:
            wkpool = ph.enter_context(tc.tile_pool(name="wkpool", bufs=1))
            wvpool = ph.enter_context(tc.tile_pool(name="wvpool", bufs=1))
            kvsb = ph.enter_context(tc.tile_pool(name="kvsb", bufs=3))
            kps_pool = ph.enter_context(tc.tile_pool(name="kps", bufs=1, space="PSUM"))
            vps_pool = ph.enter_context(tc.tile_pool(name="vps", bufs=1, space="PSUM"))

            # DMA priority order on the Sync queue: x and wk first (K proj),
            # cos/sin (rope), wv (V proj), then all of wq. Everything lands
            # before the AllGather data phase starts.
            wkts = [wkpool.tile([128, DC, 128], BF16, name=f"wkt{hk}")
                    for hk in range(HKV)]
            nc.sync.dma_start(xts[:, 0:2, :], xT_r[:, 0:2, :])
            nc.sync.dma_start(wkts[0][:, 0:4, :], wk_d[0][:, 0:4, :])
            nc.sync.dma_start(xts[:, 2:4, :], xT_r[:, 2:4, :])
            nc.sync.dma_start(wkts[0][:, 4:16, :], wk_d[0][:, 4:16, :])
            # wk1-3 before cos/sin and the x tail: the dc-major K loop needs
            # all four wk tiles from dc=0, while cos/sin are only needed at
            # the RoPE (~10us later)
            for hk in range(1, HKV):
                nc.sync.dma_start(wkts[hk][:], wk_d[hk])
            for c4 in range(1, 4):
                nc.sync.dma_start(xts[:, c4 * 4:(c4 + 1) * 4, :],
                                  xT_r[:, c4 * 4:(c4 + 1) * 4, :])
            nc.sync.dma_start(cos_sb[:], cosT_d)
            nc.sync.dma_start(sin_sb[:], sinT_d)
            wvts = []
            wv_r = wv_d.rearrange("(g d) p c -> g d p c", d=4)
            for g4 in range(4):
                wvt = wvpool.tile([128, 4, HKV * HD], BF16, name=f"wvt{g4}")
                nc.sync.dma_start(
                    wvt[:], wv_r[g4].rearrange("d p c -> p d c"))
                wvts.append(wvt)
            wq_r = wq_d.rearrange("h p dc c -> p h (dc c)")
            for q4 in range(4):
                nc.sync.dma_start(wq_sb[:, q4 * 4:(q4 + 1) * 4, :],
                                  wq_r[:, q4 * 4:(q4 + 1) * 4, :])

            # K projection, dc-major: all 4 heads accumulate in parallel
            # PSUM banks, matmuls start on the first x/wk chunks
            kps_tiles = [kps_pool.tile([128, NQ], F32, name=f"kps{hk}")
                         for hk in range(HKV)]
            for dc in range(DC):
                for hk in range(HKV):
                    nc.tensor.matmul(kps_tiles[hk][:], wkts[hk][:, dc, :],
                                     xts[:, dc, :],
                                     start=(dc == 0), stop=(dc == DC - 1))
            for hk in range(HKV):
                _rope(nc, tmp_pool, k_loc[:, hk, :], kps_tiles[hk],
                      cos_sb, sin_sb)
                nc.gpsimd.dma_start(kv_bounce[hk * 128:(hk + 1) * 128, :],
                                    k_loc[:, hk, :])

            # n4-major so each vps accumulator finishes early and its DVE
            # drain (bounce cast + vp_loc copies) overlaps the remaining
            # V-proj matmuls -- otherwise Q-proj waits ~10us on the PSUM
            # bank handoff at the end
            vps_tiles = [vps_pool.tile([128, HKV * HD], F32, name=f"vps{i}")
                         for i in range(4)]
            for n4 in range(4):
                for g4 in range(4):
                    for d4 in range(4):
                        dc = g4 * 4 + d4
                        nc.tensor.matmul(
                            vps_tiles[n4][:],
                            xts[:, dc, n4 * 128:(n4 + 1) * 128],
                            wvts[g4][:, d4, :],
                            start=(dc == 0), stop=(dc == DC - 1))
                v_sb = kvsb.tile([128, HKV * HD], BF16, name="v_sb")
                nc.vector.tensor_copy(v_sb[:], vps_tiles[n4][:])
                nc.gpsimd.dma_start(
                    kv_bounce[NQ + n4 * 128:NQ + (n4 + 1) * 128, :], v_sb[:])
                for hk in range(HKV):
                    nc.vector.tensor_copy(
                        vp_loc[:, hk * 4 + n4, 0:HD],
                        vps_tiles[n4][:, hk * HD:(hk + 1) * HD])

        # ---------------- AllGather K,V within the batch group ----------------
        nc.gpsimd.collective_compute(
            "AllGather", mybir.AluOpType.bypass,
            replica_groups=[[0, 1, 2, 3], [4, 5, 6, 7]],
            ins=[kv_bounce.opt()],
            outs=[ag_out.opt()],
        )

        # ---------------- fused Q projection + RoPE + stage L -----------------
        # per iteration i: [Qproj head i+1 | scores head i | AV head i-1] on
        # the PE; ScalarE does rope copies of head i+1 then exp of head i,
        # both far off the critical path.
        last_l_mm = None
        with ExitStack() as ph:
            stl_pool = ph.enter_context(tc.tile_pool(name="stl", bufs=1, space="PSUM"))
            qps_pool = ph.enter_context(tc.tile_pool(name="qps", bufs=2, space="PSUM"))
            opsl_pool = ph.enter_context(tc.tile_pool(name="opsl", bufs=2, space="PSUM"))

            def qproj(h):
                qps = qps_pool.tile([128, NQ], F32, name="qps_t")
                for dc in range(DC):
                    nc.tensor.matmul(qps[:],
                                     wq_sb[:, h, dc * 128:(dc + 1) * 128],
                                     xts[:, dc, :],
                                     start=(dc == 0), stop=(dc == DC - 1))
                _rope(nc, tmp_pool, q_sb[:, h, :], qps, cos_sb, sin_sb)

            qproj(0)
            pend = {}    # head -> pt tile [128, 4, NQ]
            for i in range(H + 1):
                if i + 1 <= H - 1:
                    qproj(i + 1)
                if i <= H - 1:
                    h, hk = i, i % HKV
                    st = stl_pool.tile([128, 4, NQ], F32, name="stl_t")
                    for kb in range(4):
                        nc.tensor.matmul(
                            st[:, kb, :],
                            k_loc[:, hk, kb * 128:(kb + 1) * 128],
                            q_sb[:, h, :], start=True, stop=True)
                    pt = ptpool.tile([128, 4, NQ], BF16, name="pt_t")
                    nc.scalar.activation(pt[:], st[:], EXP, scale=SCALE)
                    pend[i] = pt
                if i - 1 in pend:
                    # AV for head i-1 in two qc sweeps so the two PSUM
                    # accumulator banks (bufs=2) are reused within the iter
                    hp, hkp = i - 1, (i - 1) % HKV
                    pt = pend.pop(hp)
                    for q2 in range(2):
                        ops = [opsl_pool.tile([128, HD + 1], F32, name="opsl_t")
                               for _ in range(2)]
                        for kb in range(4):
                            for qi in range(2):
                                qc = q2 * 2 + qi
                                last_l_mm = nc.tensor.matmul(
                                    ops[qi][:],
                                    pt[:, kb, qc * 128:(qc + 1) * 128],
                                    vp_loc[:, hkp * 4 + kb, :],
                                    start=(kb == 0), stop=(kb == 3))
                        for qi in range(2):
                            qc = q2 * 2 + qi
                            nc.vector.tensor_copy(
                                o_part[:, hp * 4 + qc, :], ops[qi][:])
        proj_scope.close()

        # late-lifetime tensors: reuse the Wq/x bytes freed by the proj scope
        lsb = top.enter_context(tc.tile_pool(name="lsb", bufs=1))
        oT_sb = lsb.tile([128, H * 4, 128], BF16)
        wo_sb = lsb.tile([128, H, D], BF16)

        # ---------------- stage R: remote three rank slots --------------------
        # flat pipeline of 96 kb2-windows across all 16 heads: the ScalarE
        # exp chain never starves (window k's AV burst runs at window k+2,
        # which may belong to the next head), and one head's four PSUM
        # accumulator banks recycle to the next head right after its norms.
        with ExitStack() as ph:
            ptr_pool = ph.enter_context(tc.tile_pool(name="ptr", bufs=6))
            nr_pool = ph.enter_context(tc.tile_pool(name="nr", bufs=6))
            str_pool = ph.enter_context(tc.tile_pool(name="str", bufs=2, space="PSUM"))
            opsr_pool = ph.enter_context(tc.tile_pool(name="opsr", bufs=4, space="PSUM"))

            pid = nc.sync.partition_id()
            slots = [(pid + i) % 4 for i in (1, 2, 3)]
            first_r_mm = None
            ops_by_head = {}
            pending = []

            def norm_out(h, qc, ops_q):
                of = nr_pool.tile([128, HD + 1], F32, name="of")
                nc.vector.tensor_tensor(
                    of[:], ops_q[:], o_part[:, h * 4 + qc, :], ADD)
                rin = nr_pool.tile([128, 1], F32, name="rin")
                nc.vector.reciprocal(rin[:], of[:, HD:HD + 1])
                o_n = nr_pool.tile([128, 128], BF16, name="o_n")
                nc.vector.tensor_scalar_mul(o_n[:], of[:, 0:HD], rin[:])
                nc.sync.dma_start(oT_sb[:, h * 4 + qc, :], o_n[:],
                                  transpose=True)

            def do_burst(h, w, pt, hk_b):
                if w == 0:
                    ops_by_head[h] = [
                        opsr_pool.tile([128, HD + 1], F32, name="opsr_t")
                        for _ in range(4)]
                ops = ops_by_head[h]
                if w < 5:
                    for j in range(2):
                        rb = 2 * w + j
                        for qc in range(4):
                            nc.tensor.matmul(
                                ops[qc][:], pt[:, j, qc * 128:(qc + 1) * 128],
                                vpr_all[:, rb // 4, rb % 4, hk_b, :],
                                start=(rb == 0), stop=(rb == 11))
                else:
                    # last window: finish each qc then norm it right away so
                    # the banks free up for the next head with minimal WAR
                    for qc in range(4):
                        for j in range(2):
                            rb = 2 * w + j
                            nc.tensor.matmul(
                                ops[qc][:], pt[:, j, qc * 128:(qc + 1) * 128],
                                vpr_all[:, rb // 4, rb % 4, hk_b, :],
                                start=(rb == 0), stop=(rb == 11))
                        norm_out(h, qc, ops[qc])
                    del ops_by_head[h]

            for i, slot in enumerate(slots):
                for kbl in range(4):
                    src = ag_out[bass.ds(slot * 2 * NQ + NQ + kbl * 128, 128), :]
                    nc.sync.dma_start(
                        vpr_all[:, i, kbl, :, 0:HD],
                        src.rearrange("p (hk c) -> p hk c", c=HD))
            for hk in range(HKV):
                ktr = ktpool.tile([128, 3, NQ], BF16, name="ktr")
                for i, slot in enumerate(slots):
                    nc.sync.dma_start(
                        ktr[:, i, :],
                        ag_out[bass.ds(slot * 2 * NQ + hk * 128, 128), :])
                if hk == 3:
                    # Wo prefetch: lands long before the output projection
                    nc.sync.dma_start(wo_sb[:], wo_d.rearrange("h p c -> p h c"))

                for g in range(4):
                    h = g * HKV + hk
                    for w in range(6):
                        st = str_pool.tile([128, 2, NQ], F32, name="str_t")
                        for j in range(2):
                            rb = 2 * w + j
                            mm = nc.tensor.matmul(
                                st[:, j, :],
                                ktr[:, rb // 4, (rb % 4) * 128:(rb % 4 + 1) * 128],
                                q_sb[:, h, :], start=True, stop=True)
                            if first_r_mm is None:
                                first_r_mm = mm
                                tile.add_dep_helper(
                                    first_r_mm.ins, last_l_mm.ins,
                                    reason="stage R after stage L (PE order)")
                        pt = ptr_pool.tile([128, 2, NQ], BF16, name="ptr_t")
                        nc.scalar.activation(pt[:], st[:], EXP, scale=SCALE)
                        pending.append((h, w, pt, hk))
                        if len(pending) > 2:
                            do_burst(*pending.pop(0))
            while pending:
                do_burst(*pending.pop(0))

        # ---------------- output projection -----------------------------------
        with ExitStack() as ph:
            outsb = ph.enter_context(tc.tile_pool(name="outsb", bufs=4))
            outps = ph.enter_context(tc.tile_pool(name="outps", bufs=3, space="PSUM"))
            for dcol in range(4):
                for qc in range(4):
                    outp = outps.tile([128, 512], F32, name="outp")
                    for hh in range(H):
                        nc.tensor.matmul(
                            outp[:], oT_sb[:, hh * 4 + qc, :],
                            wo_sb[:, hh, dcol * 512:(dcol + 1) * 512],
                            start=(hh == 0), stop=(hh == H - 1))
                    osb = outsb.tile([128, 512], F32, name="osb")
                    nc.vector.tensor_copy(osb[:], outp[:])
                    nc.sync.dma_start(
                        out_d[qc * 128:(qc + 1) * 128,
                              dcol * 512:(dcol + 1) * 512], osb[:])

    nc.compile()
    return nc


def _prep_inputs(x, cos, sin, Wq, Wkv, Wo):
    bf = ml_dtypes.bfloat16
    wq_prep = np.ascontiguousarray(
        Wq.reshape(DC, 128, H, HD).transpose(2, 1, 0, 3)).astype(bf)
    wk_prep = np.ascontiguousarray(
        Wkv[:, :HKV * HD].reshape(DC, 128, HKV, HD).transpose(2, 1, 0, 3)).astype(bf)
    wv_prep = np.ascontiguousarray(
        Wkv[:, HKV * HD:].reshape(DC, 128, HKV * HD)).astype(bf)
    wo_prep = np.ascontiguousarray(Wo.reshape(H, HD, D)).astype(bf)
    c64 = cos[0, :, 0, :].T.astype(np.float32)   # [64, N]
    s64 = sin[0, :, 0, :].T.astype(np.float32)
    cosT = np.ascontiguousarray(np.concatenate([c64, c64], axis=0))   # [128, N]
    sinT = np.ascontiguousarray(np.concatenate([s64, -s64], axis=0))

    in_maps = []
    for c in range(NCORES):
        b, j = divmod(c, 4)
        rows = slice(j * NQ, (j + 1) * NQ)
        xT = np.ascontiguousarray(x[b].T[:, rows]).astype(bf)
        in_maps.append({
            "xT": xT,
            "cosT": np.ascontiguousarray(cosT[:, rows]),
            "sinT": np.ascontiguousarray(sinT[:, rows]),
            "wq": wq_prep, "wk": wk_prep, "wv": wv_prep, "wo": wo_prep,
        })
    return in_maps


def kernel(x, cos, sin, attn_mask, Wq, Wkv, Wo, bo):
    x = np.asarray(x, np.float32)
    cos = np.asarray(cos, np.float32)
    sin = np.asarray(sin, np.float32)
    Wq = np.asarray(Wq, np.float32)
    Wkv = np.asarray(Wkv, np.float32)
    Wo = np.asarray(Wo, np.float32)
    bo = np.asarray(bo, np.float32)

    if "nc" not in _cache:
        _cache["nc"] = _build()
    nc = _cache["nc"]

    in_maps = _prep_inputs(x, cos, sin, Wq, Wkv, Wo)
    res = run_bass_kernel_spmd(nc, in_maps, list(range(NCORES)))
    out = np.empty((B, N, D), np.float32)
    for c in range(NCORES):
        b, j = divmod(c, 4)
        out[b, j * NQ:(j + 1) * NQ, :] = res.results[c]["out"]
    out += bo[None, None, :]
    return out


# revision 27
# speedup vs baseline: 1.0054x; 1.0054x over previous
"""GQA attention block (B=2, N=2048, D=2048, H=16, HKV=4, HD=128) on 8 TRN2 cores.

Sharding: core c -> batch b = c // 4, query-row quarter j = c % 4 (512 rows).
Each core:
  - projects K,V for its row slice (K projection dc-major so matmuls start as
    soon as the first weight chunks land), applies RoPE to K, AllGathers K,V
    within its 4-core batch group (single 1MB collective). The bounce-buffer
    writes are issued from the GpSimd SWDGE queue so they neither wait behind
    the 8MB Wq load on the Sync queue nor block the RoPE copies on the Scalar
    queue -- this fires the AllGather ~30us earlier.
  - Q projection is FUSED with the local-attention stage L: each iteration
    runs [Qproj head i+1 | scores head i | AV head i-1] on the PE, so the
    ScalarE exp and RoPE copies hide inside the 4.2us Qproj block and the
    softmax exp is never on the critical path here.
  - stage R (the three remote rank slots, read from the gathered buffer with
    rank-dependent dynamic DMA offsets, streamed one kv-head ahead): scores
    in groups of THREE key blocks per exp activation (amortizes the ~300
    cycle ACT fixed cost) and the AV burst lags TWO windows behind the
    scores, keeping both semaphore round trips off the critical loop. This
    makes stage R tensor-bound instead of exp-bound.
  - softmax denominator via a ones-column appended to V; per-query
    normalization on DVE; o -> o^T via DMA xbar transpose issued per head
    (no PE cycles, no cold-clock window from a sparse transpose phase).
  - output projection over all heads -> its own 512 output rows (no reduce);
    Wo is prefetched during stage R.
All matmuls bf16 with f32 PSUM accumulation; softmax statistics in f32.
"""

import numpy as np
import ml_dtypes

import concourse.bass as bass
import concourse.mybir as mybir
import concourse.tile as tile
from concourse import bacc, masks
from concourse.bass_utils import run_bass_kernel_spmd

B, N, D = 2, 2048, 2048
H, HKV, HD = 16, 4, 128
NQ = N // 4          # query rows per core
DC = D // 128        # contraction chunks for projections
KB = N // 128        # key blocks
NCORES = 8
SCALE = float(HD) ** -0.5

BF16 = mybir.dt.bfloat16
F32 = mybir.dt.float32
MUL = mybir.AluOpType.mult
ADD = mybir.AluOpType.add
EXP = mybir.ActivationFunctionType.Exp

_cache = {}


def _rope(nc, pool, out, in_psum, cos2_sb, sin2_sb):
    """Rotate-half RoPE with head-dim on partitions.

    cos2_sb = [cos; cos], sin2_sb = [sin; -sin] (128 rows, host-prepared), so
    out = t*cos2 + rot(t)*sin2 where rot swaps the partition halves.
    ScalarE does the PSUM reads; the three DVE multiplies/adds then run
    all-SBUF at the 2x f32 rate.
    """
    rot = pool.tile([128, NQ], F32, name="rope_rot")
    nc.scalar.copy(rot[0:64, :], in_psum[64:128, :])
    nc.scalar.copy(rot[64:128, :], in_psum[0:64, :])
    m1 = pool.tile([128, NQ], F32, name="rope_m1")
    m2 = pool.tile([128, NQ], F32, name="rope_m2")
    nc.vector.tensor_tensor(m1[:], in_psum[:], cos2_sb[:], MUL)
    nc.vector.tensor_tensor(m2[:], rot[:], sin2_sb[:], MUL)
    nc.vector.tensor_tensor(out[:], m1[:], m2[:], ADD)


def _build():
    from contextlib import ExitStack

    nc = bacc.Bacc("TRN2", target_bir_lowering=False, debug=False,
                   num_devices=NCORES)

    xT_d = nc.dram_tensor("xT", [D, NQ], BF16, kind="ExternalInput").ap()
    cosT_d = nc.dram_tensor("cosT", [HD, NQ], F32, kind="ExternalInput").ap()
    sinT_d = nc.dram_tensor("sinT", [HD, NQ], F32, kind="ExternalInput").ap()
    wq_d = nc.dram_tensor("wq", [H, 128, DC, 128], BF16, kind="ExternalInput").ap()
    wk_d = nc.dram_tensor("wk", [HKV, 128, DC, 128], BF16, kind="ExternalInput").ap()
    wv_d = nc.dram_tensor("wv", [DC, 128, HKV * HD], BF16, kind="ExternalInput").ap()
    wo_d = nc.dram_tensor("wo", [H, 128, D], BF16, kind="ExternalInput").ap()
    out_d = nc.dram_tensor("out", [NQ, D], F32, kind="ExternalOutput").ap()

    with tile.TileContext(nc) as tc, ExitStack() as top:
        resident = top.enter_context(tc.tile_pool(name="resident", bufs=1))
        dram = top.enter_context(tc.tile_pool(name="dram", bufs=1, space="DRAM"))

        q_sb = resident.tile([128, H, NQ], BF16)
        k_loc = resident.tile([128, HKV, NQ], BF16)       # roped local K, d-major
        vp_loc = resident.tile([128, HKV * 4, HD + 1], BF16)  # (hk, local kb)
        nc.gpsimd.memset(vp_loc[:, :, HD:HD + 1], 1.0)
        # unnormalized local partials (h, qc), col 128 = partial denominator
        o_part = resident.tile([128, H * 4, HD + 1], BF16)

        # remote K/V slab pools reserved at top level: their SBUF must NOT
        # overlap the proj-scope bytes, so the slab DMAs can start the moment
        # the AllGather completes (during the fused Q+L phase)
        ktpool = top.enter_context(tc.tile_pool(name="ktpool", bufs=3))
        # all remote V, loaded in 3 per-slot batched reads (one per rank slot,
        # covering all 4 kv-heads: 4x fewer DMA descriptors to generate)
        vpr_all = top.enter_context(tc.tile_pool(name="vpra", bufs=1)).tile(
            [128, 3, 4, HKV, HD + 1], BF16)
        nc.gpsimd.memset(vpr_all[:, :, :, :, HD:HD + 1], 1.0)

        # kv bounce: rows 0..511 = roped K (4 heads x 128 d), cols = local n;
        # rows 512..1023 = V (local n rows), cols = 4 heads x 128 channels
        kv_bounce = dram.tile([2 * NQ, NQ], BF16)
        ag_out = dram.tile([2 * NQ * 4, NQ], BF16)

        # -- projection scope: tensors freed after the fused Q+L phase --------
        proj_scope = ExitStack()
        proj = proj_scope.enter_context(tc.tile_pool(name="proj", bufs=1))
        tmp_pool = proj_scope.enter_context(tc.tile_pool(name="ropetmp", bufs=3))
        cos_sb = proj.tile([HD, NQ], F32)
        sin_sb = proj.tile([HD, NQ], F32)
        xts = proj.tile([128, DC, NQ], BF16)
        wq_sb = proj.tile([128, H, DC * 128], BF16)
        xT_r = xT_d.rearrange("(dc p) n -> p dc n", p=128)

        # pt/npool shared by stage L and stage R; entered after the proj pools
        # but they only hold small streaming tiles
        ptpool = proj_scope.enter_context(tc.tile_pool(name="ptpool", bufs=2))
        npool = proj_scope.enter_context(tc.tile_pool(name="npool", bufs=6))

        # ---------------- KV projection + RoPE(K) + bounce-out ----------------
        with ExitStack() as ph:
            wkpool = ph.enter_context(tc.tile_pool(name="wkpool", bufs=1))
            wvpool = ph.enter_context(tc.tile_pool(name="wvpool", bufs=1))
            kvsb = ph.enter_context(tc.tile_pool(name="kvsb", bufs=3))
            kps_pool = ph.enter_context(tc.tile_pool(name="kps", bufs=1, space="PSUM"))
            vps_pool = ph.enter_context(tc.tile_pool(name="vps", bufs=1, space="PSUM"))

            # DMA priority order on the Sync queue: x and wk first (K proj),
            # cos/sin (rope), wv (V proj), then all of wq. Everything lands
            # before the AllGather data phase starts.
            wkts = [wkpool.tile([128, DC, 128], BF16, name=f"wkt{hk}")
                    for hk in range(HKV)]
            nc.sync.dma_start(xts[:, 0:2, :], xT_r[:, 0:2, :])
            nc.sync.dma_start(wkts[0][:, 0:4, :], wk_d[0][:, 0:4, :])
            nc.sync.dma_start(xts[:, 2:4, :], xT_r[:, 2:4, :])
            nc.sync.dma_start(wkts[0][:, 4:16, :], wk_d[0][:, 4:16, :])
            nc.sync.dma_start(cos_sb[:], cosT_d)
            nc.sync.dma_start(sin_sb[:], sinT_d)
            for c4 in range(1, 4):
                nc.sync.dma_start(xts[:, c4 * 4:(c4 + 1) * 4, :],
                                  xT_r[:, c4 * 4:(c4 + 1) * 4, :])
            for hk in range(1, HKV):
                nc.sync.dma_start(wkts[hk][:], wk_d[hk])
            wvts = []
            wv_r = wv_d.rearrange("(g d) p c -> g d p c", d=4)
            for g4 in range(4):
                wvt = wvpool.tile([128, 4, HKV * HD], BF16, name=f"wvt{g4}")
                nc.sync.dma_start(
                    wvt[:], wv_r[g4].rearrange("d p c -> p d c"))
                wvts.append(wvt)
            wq_r = wq_d.rearrange("h p dc c -> p h (dc c)")
            for q4 in range(4):
                nc.sync.dma_start(wq_sb[:, q4 * 4:(q4 + 1) * 4, :],
                                  wq_r[:, q4 * 4:(q4 + 1) * 4, :])

            # K projection, dc-major: all 4 heads accumulate in parallel
            # PSUM banks, matmuls start on the first x/wk chunks
            kps_tiles = [kps_pool.tile([128, NQ], F32, name=f"kps{hk}")
                         for hk in range(HKV)]
            for dc in range(DC):
                for hk in range(HKV):
                    nc.tensor.matmul(kps_tiles[hk][:], wkts[hk][:, dc, :],
                                     xts[:, dc, :],
                                     start=(dc == 0), stop=(dc == DC - 1))
            for hk in range(HKV):
                _rope(nc, tmp_pool, k_loc[:, hk, :], kps_tiles[hk],
                      cos_sb, sin_sb)
                nc.gpsimd.dma_start(kv_bounce[hk * 128:(hk + 1) * 128, :],
                                    k_loc[:, hk, :])

            # n4-major so each vps accumulator finishes early and its DVE
            # drain (bounce cast + vp_loc copies) overlaps the remaining
            # V-proj matmuls -- otherwise Q-proj waits ~10us on the PSUM
            # bank handoff at the end
            vps_tiles = [vps_pool.tile([128, HKV * HD], F32, name=f"vps{i}")
                         for i in range(4)]
            for n4 in range(4):
                for g4 in range(4):
                    for d4 in range(4):
                        dc = g4 * 4 + d4
                        nc.tensor.matmul(
                            vps_tiles[n4][:],
                            xts[:, dc, n4 * 128:(n4 + 1) * 128],
                            wvts[g4][:, d4, :],
                            start=(dc == 0), stop=(dc == DC - 1))
                v_sb = kvsb.tile([128, HKV * HD], BF16, name="v_sb")
                nc.vector.tensor_copy(v_sb[:], vps_tiles[n4][:])
                nc.gpsimd.dma_start(
                    kv_bounce[NQ + n4 * 128:NQ + (n4 + 1) * 128, :], v_sb[:])
                for hk in range(HKV):
                    nc.vector.tensor_copy(
                        vp_loc[:, hk * 4 + n4, 0:HD],
                        vps_tiles[n4][:, hk * HD:(hk + 1) * HD])

        # ---------------- AllGather K,V within the batch group ----------------
        nc.gpsimd.collective_compute(
            "AllGather", mybir.AluOpType.bypass,
            replica_groups=[[0, 1, 2, 3], [4, 5, 6, 7]],
            ins=[kv_bounce.opt()],
            outs=[ag_out.opt()],
        )

        # ---------------- fused Q projection + RoPE + stage L -----------------
        # per iteration i: [Qproj head i+1 | scores head i | AV head i-1] on
        # the PE; ScalarE does rope copies of head i+1 then exp of head i,
        # both far off the critical path.
        last_l_mm = None
        with ExitStack() as ph:
            stl_pool = ph.enter_context(tc.tile_pool(name="stl", bufs=1, space="PSUM"))
            qps_pool = ph.enter_context(tc.tile_pool(name="qps", bufs=2, space="PSUM"))
            opsl_pool = ph.enter_context(tc.tile_pool(name="opsl", bufs=2, space="PSUM"))

            def qproj(h):
                qps = qps_pool.tile([128, NQ], F32, name="qps_t")
                for dc in range(DC):
                    nc.tensor.matmul(qps[:],
                                     wq_sb[:, h, dc * 128:(dc + 1) * 128],
                                     xts[:, dc, :],
                                     start=(dc == 0), stop=(dc == DC - 1))
                _rope(nc, tmp_pool, q_sb[:, h, :], qps, cos_sb, sin_sb)

            qproj(0)
            pend = {}    # head -> pt tile [128, 4, NQ]
            for i in range(H + 1):
                if i + 1 <= H - 1:
                    qproj(i + 1)
                if i <= H - 1:
                    h, hk = i, i % HKV
                    st = stl_pool.tile([128, 4, NQ], F32, name="stl_t")
                    for kb in range(4):
                        nc.tensor.matmul(
                            st[:, kb, :],
                            k_loc[:, hk, kb * 128:(kb + 1) * 128],
                            q_sb[:, h, :], start=True, stop=True)
                    pt = ptpool.tile([128, 4, NQ], BF16, name="pt_t")
                    nc.scalar.activation(pt[:], st[:], EXP, scale=SCALE)
                    pend[i] = pt
                if i - 1 in pend:
                    # AV for head i-1 in two qc sweeps so the two PSUM
                    # accumulator banks (bufs=2) are reused within the iter
                    hp, hkp = i - 1, (i - 1) % HKV
                    pt = pend.pop(hp)
                    for q2 in range(2):
                        ops = [opsl_pool.tile([128, HD + 1], F32, name="opsl_t")
                               for _ in range(2)]
                        for kb in range(4):
                            for qi in range(2):
                                qc = q2 * 2 + qi
                                last_l_mm = nc.tensor.matmul(
                                    ops[qi][:],
                                    pt[:, kb, qc * 128:(qc + 1) * 128],
                                    vp_loc[:, hkp * 4 + kb, :],
                                    start=(kb == 0), stop=(kb == 3))
                        for qi in range(2):
                            qc = q2 * 2 + qi
                            nc.vector.tensor_copy(
                                o_part[:, hp * 4 + qc, :], ops[qi][:])
        proj_scope.close()

        # late-lifetime tensors: reuse the Wq/x bytes freed by the proj scope
        lsb = top.enter_context(tc.tile_pool(name="lsb", bufs=1))
        oT_sb = lsb.tile([128, H * 4, 128], BF16)
        wo_sb = lsb.tile([128, H, D], BF16)

        # ---------------- stage R: remote three rank slots --------------------
        # flat pipeline of 96 kb2-windows across all 16 heads: the ScalarE
        # exp chain never starves (window k's AV burst runs at window k+2,
        # which may belong to the next head), and one head's four PSUM
        # accumulator banks recycle to the next head right after its norms.
        with ExitStack() as ph:
            ptr_pool = ph.enter_context(tc.tile_pool(name="ptr", bufs=6))
            nr_pool = ph.enter_context(tc.tile_pool(name="nr", bufs=6))
            str_pool = ph.enter_context(tc.tile_pool(name="str", bufs=2, space="PSUM"))
            opsr_pool = ph.enter_context(tc.tile_pool(name="opsr", bufs=4, space="PSUM"))

            pid = nc.sync.partition_id()
            slots = [(pid + i) % 4 for i in (1, 2, 3)]
            first_r_mm = None
            ops_by_head = {}
            pending = []

            def norm_out(h, qc, ops_q):
                of = nr_pool.tile([128, HD + 1], F32, name="of")
                nc.vector.tensor_tensor(
                    of[:], ops_q[:], o_part[:, h * 4 + qc, :], ADD)
                rin = nr_pool.tile([128, 1], F32, name="rin")
                nc.vector.reciprocal(rin[:], of[:, HD:HD + 1])
                o_n = nr_pool.tile([128, 128], BF16, name="o_n")
                nc.vector.tensor_scalar_mul(o_n[:], of[:, 0:HD], rin[:])
                nc.sync.dma_start(oT_sb[:, h * 4 + qc, :], o_n[:],
                                  transpose=True)

            def do_burst(h, w, pt, hk_b):
                if w == 0:
                    ops_by_head[h] = [
                        opsr_pool.tile([128, HD + 1], F32, name="opsr_t")
                        for _ in range(4)]
                ops = ops_by_head[h]
                if w < 5:
                    for j in range(2):
                        rb = 2 * w + j
                        for qc in range(4):
                            nc.tensor.matmul(
                                ops[qc][:], pt[:, j, qc * 128:(qc + 1) * 128],
                                vpr_all[:, rb // 4, rb % 4, hk_b, :],
                                start=(rb == 0), stop=(rb == 11))
                else:
                    # last window: finish each qc then norm it right away so
                    # the banks free up for the next head with minimal WAR
                    for qc in range(4):
                        for j in range(2):
                            rb = 2 * w + j
                            nc.tensor.matmul(
                                ops[qc][:], pt[:, j, qc * 128:(qc + 1) * 128],
                                vpr_all[:, rb // 4, rb % 4, hk_b, :],
                                start=(rb == 0), stop=(rb == 11))
                        norm_out(h, qc, ops[qc])
                    del ops_by_head[h]

            for i, slot in enumerate(slots):
                for kbl in range(4):
                    src = ag_out[bass.ds(slot * 2 * NQ + NQ + kbl * 128, 128), :]
                    nc.sync.dma_start(
                        vpr_all[:, i, kbl, :, 0:HD],
                        src.rearrange("p (hk c) -> p hk c", c=HD))
            for hk in range(HKV):
                ktr = ktpool.tile([128, 3, NQ], BF16, name="ktr")
                for i, slot in enumerate(slots):
                    nc.sync.dma_start(
                        ktr[:, i, :],
                        ag_out[bass.ds(slot * 2 * NQ + hk * 128, 128), :])
                if hk == 3:
                    # Wo prefetch: lands long before the output projection
                    nc.sync.dma_start(wo_sb[:], wo_d.rearrange("h p c -> p h c"))

                for g in range(4):
                    h = g * HKV + hk
                    for w in range(6):
                        st = str_pool.tile([128, 2, NQ], F32, name="str_t")
                        for j in range(2):
                            rb = 2 * w + j
                            mm = nc.tensor.matmul(
                                st[:, j, :],
                                ktr[:, rb // 4, (rb % 4) * 128:(rb % 4 + 1) * 128],
                                q_sb[:, h, :], start=True, stop=True)
                            if first_r_mm is None:
                                first_r_mm = mm
                                tile.add_dep_helper(
                                    first_r_mm.ins, last_l_mm.ins,
                                    reason="stage R after stage L (PE order)")
                        pt = ptr_pool.tile([128, 2, NQ], BF16, name="ptr_t")
                        nc.scalar.activation(pt[:], st[:], EXP, scale=SCALE)
                        pending.append((h, w, pt, hk))
                        if len(pending) > 2:
                            do_burst(*pending.pop(0))
            while pending:
                do_burst(*pending.pop(0))

        # ---------------- output projection -----------------------------------
        with ExitStack() as ph:
            outsb = ph.enter_context(tc.tile_pool(name="outsb", bufs=4))
            outps = ph.enter_context(tc.tile_pool(name="outps", bufs=3, space="PSUM"))
            for dcol in range(4):
                for qc in range(4):
                    outp = outps.tile([128, 512], F32, name="outp")
                    for hh in range(H):
                        nc.tensor.matmul(
                            outp[:], oT_sb[:, hh * 4 + qc, :],
                            wo_sb[:, hh, dcol * 512:(dcol + 1) * 512],
                            start=(hh == 0), stop=(hh == H - 1))
                    osb = outsb.tile([128, 512], F32, name="osb")
                    nc.vector.tensor_copy(osb[:], outp[:])
                    nc.sync.dma_start(
                        out_d[qc * 128:(qc + 1) * 128,
                              dcol * 512:(dcol + 1) * 512], osb[:])

    nc.compile()
    return nc


def _prep_inputs(x, cos, sin, Wq, Wkv, Wo):
    bf = ml_dtypes.bfloat16
    wq_prep = np.ascontiguousarray(
        Wq.reshape(DC, 128, H, HD).transpose(2, 1, 0, 3)).astype(bf)
    wk_prep = np.ascontiguousarray(
        Wkv[:, :HKV * HD].reshape(DC, 128, HKV, HD).transpose(2, 1, 0, 3)).astype(bf)
    wv_prep = np.ascontiguousarray(
        Wkv[:, HKV * HD:].reshape(DC, 128, HKV * HD)).astype(bf)
    wo_prep = np.ascontiguousarray(Wo.reshape(H, HD, D)).astype(bf)
    c64 = cos[0, :, 0, :].T.astype(np.float32)   # [64, N]
    s64 = sin[0, :, 0, :].T.astype(np.float32)
    cosT = np.ascontiguousarray(np.concatenate([c64, c64], axis=0))   # [128, N]
    sinT = np.ascontiguousarray(np.concatenate([s64, -s64], axis=0))

    in_maps = []
    for c in range(NCORES):
        b, j = divmod(c, 4)
        rows = slice(j * NQ, (j + 1) * NQ)
        xT = np.ascontiguousarray(x[b].T[:, rows]).astype(bf)
        in_maps.append({
            "xT": xT,
            "cosT": np.ascontiguousarray(cosT[:, rows]),
            "sinT": np.ascontiguousarray(sinT[:, rows]),
            "wq": wq_prep, "wk": wk_prep, "wv": wv_prep, "wo": wo_prep,
        })
    return in_maps


def kernel(x, cos, sin, attn_mask, Wq, Wkv, Wo, bo):
    x = np.asarray(x, np.float32)
    cos = np.asarray(cos, np.float32)
    sin = np.asarray(sin, np.float32)
    Wq = np.asarray(Wq, np.float32)
    Wkv = np.asarray(Wkv, np.float32)
    Wo = np.asarray(Wo, np.float32)
    bo = np.asarray(bo, np.float32)

    if "nc" not in _cache:
        _cache["nc"] = _build()
    nc = _cache["nc"]

    in_maps = _prep_inputs(x, cos, sin, Wq, Wkv, Wo)
    res = run_bass_kernel_spmd(nc, in_maps, list(range(NCORES)))
    out = np.empty((B, N, D), np.float32)
    for c in range(NCORES):
        b, j = divmod(c, 4)
        out[b, j * NQ:(j + 1) * NQ, :] = res.results[c]["out"]
    out += bo[None, None, :]
    return out
